# revision 1
# baseline (speedup 1.0000x reference)
"""DiffAttn TRN2 kernel: 8-core SPMD (batch x query-half sharding).

Algebraic restructure vs the direct formulation:

1. Fold the q/k projections into M_a = Wq_a @ Wk_a^T (host, [D, 2D]):
     scores_a = (xq @ M_a) @ x^T          (A-matmul + scores matmul)
   removing the k-projection and all K staging through DRAM.
2. Reassociate the output matmul:
     out = diff @ (x @ Wv) = (diff @ x) @ Wv
   removing the per-core-duplicated v-projection (stage2 t @ Wv is
   only QH*D*D vs the v-projection's S*D*D).
3. The second softmax's lambda weighting uses the per-query ratio
   c[q] = lam * den0[q] / den1[q]:  diff * den0 = e0 - c * e1, and the
   leading 1/den0 is skipped entirely: RMSNorm is scale-invariant per
   row (den0 > 0 so no sign flip). lam is folded on host.
4. Denominators come from an all-ones [128,128] stationary matmul, so
   den lands in PSUM already broadcast across partitions; c is then
   two DVE ops (recip + fused (recip*lam)*den0 scalar_tensor_tensor).

Per core (batch b = core//2, query half h = core%2), x rows permuted
so the core's own q-half comes first (s-order permutation is harmless:
scores/stage1 iterate s-tiles consistently; q rows map to out rows).

  phase 1: A12T[d,q] = sum_din M12[din,d] xqT[din,q]       (f32r)
  phase 3: sT_a[s,q] = sum_d xT[d,s] A_aT[d,q] (xT streamed from HBM
           as host-pre-tiled 4KB-line st-tiles); e_a = exp(scale*s)
           den_a broadcast via ones-matmul; e0 -= c*e1 (DVE bf16)
           stage1: tT[d',q] = sum_s xn[s,d'] e0[s,q]   (bf16, 2
                   concurrent PSUM chains so PE tracks the DVE stream)
           stage2: out[q,d] = sum_d' tT[d',q] wv[d',d] (bf16)
           RMSNorm * (1 - lambda_init)

SBUF: A12T 64K/part resident; x-natural bf16 32K + Wv bf16 16K
resident; e-bf16 32K per q-block; xT st-tiles streamed (24K ring).
~38MB HBM traffic per core, all overlapped.
"""

import sys

for _p in ("/opt/trn_rl_repo", "/root/.axon_site/_ro/trn_rl_repo"):
    if _p not in sys.path:
        sys.path.append(_p)

import numpy as np

import concourse.bass as bass
import concourse.mybir as mybir
from concourse import bacc
from concourse.bass_utils import run_bass_kernel_spmd
from concourse.tile import TileContext

F32 = mybir.dt.float32
F32R = mybir.dt.float32r
BF16 = mybir.dt.bfloat16
U16 = mybir.dt.uint16
AF = mybir.ActivationFunctionType
ALU = mybir.AluOpType

D = 1024          # embed dim
S = 2048          # sequence length
B = 4             # batch
NCORES = 8
QH = 1024         # query rows per core (half a sequence)
QB = 512          # query block (matmul moving dim)
NQB = QH // QB    # 2
NQT = QB // 128   # 4 q-tiles per block
NDT = D // 128    # 8 contraction tiles
NST = S // 128    # 16 key tiles
NMC = 8           # m12 column chunks streamed
MCW = 2 * D // NMC  # 256 columns per chunk
LAMBDA_INIT = 0.8
EPS = 1e-5
SCALE = float(D) ** -0.25

_CACHE = {}


def _build_nc():
    nc = bacc.Bacc("TRN2", target_bir_lowering=False, debug=False,
                   num_devices=NCORES)

    m12 = nc.declare_dram_parameter("m12", [D, 2 * D], F32, isOutput=False)
    xq = nc.declare_dram_parameter("xq", [D, QH], F32, isOutput=False)
    xtt = nc.declare_dram_parameter("xtt", [NST, 128, NDT, 128], F32,
                                    isOutput=False)
    xnb = nc.declare_dram_parameter("xnb", [S, D], U16, isOutput=False)
    wvb = nc.declare_dram_parameter("wvb", [D, D], U16, isOutput=False)
    lamc = nc.declare_dram_parameter("lamc", [128, 1], F32, isOutput=False)
    out = nc.declare_dram_parameter("out", [QH, D], F32, isOutput=True)

    m12_v = m12.ap().rearrange("(dt p) e -> p dt e", p=128).bitcast(F32R)
    xq_v = xq.ap().rearrange("(dt p) q -> p dt q", p=128).bitcast(F32R)
    xtt_v = xtt.ap().bitcast(F32R)                       # [st, p, dt, s']
    xn_v = xnb.ap().rearrange("(st p) e -> p st e", p=128).bitcast(BF16)
    wv_v = wvb.ap().rearrange("(dt p) e -> p dt e", p=128).bitcast(BF16)
    out_v = out.ap().rearrange("(t p) e -> t p e", p=128)   # [8,128,D]

    with TileContext(nc) as tc:
        singles_cm = tc.tile_pool(name="singles", bufs=1)
        singles = singles_cm.__enter__()

        onesq_f = singles.tile([128, 128], F32)
        nc.vector.memset(onesq_f, 1.0)
        onesq_bf = singles.tile([128, 128], BF16)
        nc.vector.tensor_copy(onesq_bf, onesq_f)
        lam_sb = singles.tile([128, 1], F32)
        nc.sync.dma_start(out=lam_sb, in_=lamc.ap())
        eps_sb = singles.tile([128, 1], F32)
        nc.vector.memset(eps_sb, EPS)
        wsink = singles.tile([128, 1], F32)

        # ---- resident tensors --------------------------------------------
        pa12_cm = tc.tile_pool(name="pa12", bufs=1)
        pa12 = pa12_cm.__enter__()
        pxn_cm = tc.tile_pool(name="pxn", bufs=1)
        pxn = pxn_cm.__enter__()
        pwv_cm = tc.tile_pool(name="pwv", bufs=1)
        pwv = pwv_cm.__enter__()

        a12_sb = pa12.tile([128, 2 * NDT, QH], F32R)
        xn_sb = pxn.tile([128, NST, D], BF16)
        wv_sb = pwv.tile([128, NDT, D], BF16)

        # phase-1-only tensors on the right stack (freed after)
        pxq_cm = tc.tile_pool(name="pxq", bufs=1, side="right")
        pxq = pxq_cm.__enter__()
        pm_cm = tc.tile_pool(name="pm", bufs=4, side="right")
        pm = pm_cm.__enter__()
        psa_cm = tc.tile_pool(name="psa", bufs=2, space="PSUM")
        psa = psa_cm.__enter__()

        xq_sb = pxq.tile([128, NDT, QH], F32R)

        # DMA issue order = HBM arrival order. m12 chunk 0 + xq feed the
        # first A-matmul chains; xn/wv/xtt are phase-3 inputs queued later.
        mts = {}
        mts[0] = pm.tile([128, NDT, MCW], F32R, tag="m12", name="mt", bufs=4)
        nc.sync.dma_start(out=mts[0], in_=m12_v[:, :, 0:MCW])
        for dt in range(NDT):
            nc.sync.dma_start(out=xq_sb[:, dt, :], in_=xq_v[:, dt, :])
        for mc in range(1, 4):
            mts[mc] = pm.tile([128, NDT, MCW], F32R, tag="m12", name="mt",
                              bufs=4)
            nc.sync.dma_start(out=mts[mc],
                              in_=m12_v[:, :, mc * MCW:(mc + 1) * MCW])

        # warm-up chain: keep the PE busy (and its clock ramped) while the
        # xq/m12 head of the DMA queue lands; sized to the ~13us load time
        pwarm = psa.tile([128, 128], F32, name="pwarm", bufs=1)
        for _ in range(64):
            nc.tensor.matmul(pwarm, lhsT=onesq_bf, rhs=onesq_bf,
                             start=True, stop=True)
        nc.scalar.copy(onesq_f[:, 0:1], pwarm[:, 0:1])

        # ---- phase 1: A12T[d, q] -----------------------------------------
        for mc in range(NMC):
            if mc in mts:
                mt = mts[mc]
            else:
                mt = pm.tile([128, NDT, MCW], F32R, tag="m12", name="mt",
                             bufs=4)
                nc.sync.dma_start(out=mt,
                                  in_=m12_v[:, :, mc * MCW:(mc + 1) * MCW])
            for ti in range(MCW // 128):
                t = mc * (MCW // 128) + ti
                pa = psa.tile([128, QH], F32, name="pa")
                # qc-outer: each 512-half drains while the other computes,
                # so the activation engine never backlogs at phase end
                for qc in range(QH // 512):
                    for dt in range(NDT):
                        nc.tensor.matmul(
                            pa[:, qc * 512:(qc + 1) * 512],
                            lhsT=mt[:, dt, ti * 128:(ti + 1) * 128],
                            rhs=xq_sb[:, dt, qc * 512:(qc + 1) * 512],
                            start=(dt == 0), stop=(dt == NDT - 1))
                    nc.scalar.copy(a12_sb[:, t, qc * 512:(qc + 1) * 512],
                                   pa[:, qc * 512:(qc + 1) * 512])

        # fill the pool-transition bubble (pssc waits on psa's last drain)
        # with dependency-free matmuls so the PE clock stays ramped
        pwarm2 = psa.tile([128, 128], F32, name="pwarm2", bufs=1)
        for _ in range(14):
            nc.tensor.matmul(pwarm2, lhsT=onesq_bf, rhs=onesq_bf,
                             start=True, stop=True)
        nc.scalar.copy(onesq_f[:, 1:2], pwarm2[:, 0:1])

        psa_cm.__exit__(None, None, None)
        pm_cm.__exit__(None, None, None)
        pxq_cm.__exit__(None, None, None)

        # ---- phase 3: attention ------------------------------------------
        # pxtt reuses phase-1 SBUF deliberately: the pool-transition
        # barrier gates its DMAs behind phase-1 completion, so the 24MB of
        # phase-3 streams cannot steal HBM bandwidth from the phase-1
        # critical path (DMA queues execute concurrently, not in program
        # order).
        with tc.tile_pool(name="pxtt", bufs=6) as pxtt, \
             tc.tile_pool(name="eblk", bufs=1) as eblk, \
             tc.tile_pool(name="work", bufs=2) as work, \
             tc.tile_pool(name="pssc", bufs=2, space="PSUM") as pssc, \
             tc.tile_pool(name="psden", bufs=1, space="PSUM") as psden, \
             tc.tile_pool(name="psout", bufs=1, space="PSUM") as psout:
            for bi in range(NQB):
                qs = bi * QB
                eT = {}
                pden = {}
                for a in (0, 1):
                    eT[a] = eblk.tile([128, NST, QB], BF16,
                                      tag=f"e{a}", name=f"eT{a}")
                    pden[a] = psden.tile([128, QB], F32, tag=f"den{a}",
                                         name=f"pden{a}")
                for st in range(NST):
                    xt = pxtt.tile([128, NDT, 128], F32R, tag="xtt",
                                   name="xt", bufs=6)
                    nc.sync.dma_start(out=xt, in_=xtt_v[st])
                    if bi == 0 and st == 5:
                        for dt in range(NDT):
                            nc.sync.dma_start(out=wv_sb[:, dt, :],
                                              in_=wv_v[:, dt, :])
                        for st2 in range(NST):
                            nc.sync.dma_start(out=xn_sb[:, st2, :],
                                              in_=xn_v[:, st2, :])
                    for a in (0, 1):
                        psc = pssc.tile([128, QB], F32, tag="sc", name="psc")
                        for dt in range(NDT):
                            nc.tensor.matmul(
                                psc,
                                lhsT=xt[:, dt, :],
                                rhs=a12_sb[:, a * NDT + dt, qs:qs + QB],
                                start=(dt == 0), stop=(dt == NDT - 1))
                        nc.scalar.activation(eT[a][:, st, :], psc, AF.Exp,
                                             scale=SCALE)
                    # den ones-matmuls pipelined one st behind the score
                    # chains (their exps have completed by then). The
                    # all-ones [128,128] lhsT broadcasts den over all
                    # partitions for free.
                    if st > 0:
                        for a in (0, 1):
                            nc.tensor.matmul(
                                pden[a], lhsT=onesq_bf,
                                rhs=eT[a][:, st - 1, :],
                                start=(st - 1 == 0), stop=False)
                for a in (0, 1):
                    nc.tensor.matmul(pden[a], lhsT=onesq_bf,
                                     rhs=eT[a][:, NST - 1, :],
                                     start=False, stop=True)
                # fill the den -> combine dependency bubble with
                # dependency-free matmuls (keeps the PE clock ramped)
                pwf = pssc.tile([128, QB], F32, tag="sc", name="warmf")
                for _ in range(20):
                    nc.tensor.matmul(pwf[:, 0:128], lhsT=onesq_bf,
                                     rhs=onesq_bf, start=True, stop=True)
                nc.scalar.copy(wsink, pwf[:, 0:1])
                # c[q] = lam * den0[q] / den1[q]; e0 <- e0 - c*e1.
                # 1/den0 is never applied: RMSNorm cancels per-row scales.
                rden = work.tile([128, QB], F32, tag="rden", name="rden",
                                 bufs=1)
                nc.vector.reciprocal_approx_fast(rden, pden[1])
                c_bf = work.tile([128, QB], BF16, tag="cbf", name="cbf",
                                 bufs=1)
                nc.vector.scalar_tensor_tensor(
                    c_bf, rden, lam_sb, pden[0],
                    op0=ALU.mult, op1=ALU.mult)
                for st in range(NST):
                    nc.vector.tensor_mul(eT[1][:, st, :], eT[1][:, st, :],
                                         c_bf)
                    nc.vector.tensor_sub(eT[0][:, st, :], eT[0][:, st, :],
                                         eT[1][:, st, :])
                # stage1: tT[d', q] = sum_s xn[s, d'] e0[s, q], two d'
                # chains at a time (852ns/st consumption tracks the DVE
                # combine stream), 4 passes over the e tiles
                tT = work.tile([128, NDT, QB], BF16, tag="tt", name="tT",
                               bufs=1)
                # alternate PSUM tag pairs (A/B) per pass so a pass's
                # chains never wait on the previous pass's drains
                for dp in range(NDT // 2):
                    ab = "AB"[dp % 2]
                    pt = {}
                    for k in (0, 1):
                        pt[k] = psout.tile([128, QB], F32, tag=f"p{ab}{k}",
                                           name=f"pt{k}")
                    for st in range(NST):
                        for k in (0, 1):
                            dpt = dp * 2 + k
                            nc.tensor.matmul(
                                pt[k],
                                lhsT=xn_sb[:, st, dpt * 128:(dpt + 1) * 128],
                                rhs=eT[0][:, st, :],
                                start=(st == 0), stop=(st == NST - 1))
                    for k in (0, 1):
                        nc.scalar.copy(tT[:, dp * 2 + k, :], pt[k])
                # stage2: out[q, d] = sum_d' tT[d', q] wv[d', d] + RMSNorm.
                # Square runs per-half straight off PSUM (no copy on the
                # critical path); the final scale also reads PSUM directly.
                for j in range(NQT):
                    ab = "AB"[j % 2]
                    pos = {}
                    ssqs = {}
                    for dh in range(2):
                        po = psout.tile([128, 512], F32, tag=f"p{ab}{dh}",
                                        name=f"po{dh}")
                        for dpt in range(NDT):
                            nc.tensor.matmul(
                                po,
                                lhsT=tT[:, dpt, j * 128:(j + 1) * 128],
                                rhs=wv_sb[:, dpt, dh * 512:(dh + 1) * 512],
                                start=(dpt == 0), stop=(dpt == NDT - 1))
                        pos[dh] = po
                        sqv = work.tile([128, 512], BF16, tag="sq",
                                        name="sqv", bufs=1)
                        ssqs[dh] = work.tile([128, 1], F32, tag=f"ssq{dh}",
                                             name=f"ssq{dh}")
                        nc.scalar.activation(sqv, po, AF.Square,
                                             accum_out=ssqs[dh])
                    nc.scalar.add(ssqs[0], ssqs[0], ssqs[1])
                    rms = work.tile([128, 1], F32, tag="rms", name="rms")
                    nc.scalar.activation(rms, ssqs[0], AF.Sqrt,
                                         scale=1.0 / D, bias=eps_sb)
                    rr = work.tile([128, 1], F32, tag="rr", name="rr")
                    nc.vector.reciprocal(rr, rms)
                    outs = work.tile([128, D], F32, tag="outs", name="outs",
                                     bufs=3)
                    for dh in range(2):
                        nc.vector.tensor_scalar(
                            outs[:, dh * 512:(dh + 1) * 512], pos[dh], rr,
                            1.0 - LAMBDA_INIT, op0=ALU.mult, op1=ALU.mult)
                        nc.sync.dma_start(
                            out=out_v[bi * NQT + j][:, dh * 512:(dh + 1) * 512],
                            in_=outs[:, dh * 512:(dh + 1) * 512])

        pwv_cm.__exit__(None, None, None)
        pxn_cm.__exit__(None, None, None)
        pa12_cm.__exit__(None, None, None)
        singles_cm.__exit__(None, None, None)

    nc.finalize()
    return nc


def get_nc():
    if "nc" not in _CACHE:
        _CACHE["nc"] = _build_nc()
    return _CACHE["nc"]


def _to_bf16_bits(a):
    u = np.ascontiguousarray(a, dtype=np.float32).view(np.uint32)
    return (((u >> 16) + ((u >> 15) & 1)).astype(np.uint32) & 0xFFFF).astype(
        np.uint16)


def make_in_maps(x, w_q12, w_k12, w_v, lambda_q1, lambda_k1, lambda_q2,
                 lambda_k2):
    wq = np.asarray(w_q12, dtype=np.float64)
    wk = np.asarray(w_k12, dtype=np.float64)
    m1 = wq[:, :D] @ wk[:, :D].T
    m2 = wq[:, D:] @ wk[:, D:].T
    m12_ = np.ascontiguousarray(
        np.concatenate([m1, m2], axis=1).astype(np.float32))
    wvb_ = _to_bf16_bits(np.asarray(w_v, dtype=np.float32))
    lam1 = np.exp(np.float64(lambda_q1) @ np.float64(lambda_k1))
    lam2 = np.exp(np.float64(lambda_q2) @ np.float64(lambda_k2))
    lam_ = np.full((128, 1), lam1 - lam2 + LAMBDA_INIT, dtype=np.float32)
    in_maps = []
    for c in range(NCORES):
        b, h = divmod(c, 2)
        xb = np.asarray(x[b], dtype=np.float32)
        # own q-half rows first so the kernel's q columns are 0:QH
        xp = np.concatenate([xb[h * QH:(h + 1) * QH, :],
                             xb[(1 - h) * QH:(2 - h) * QH, :]], axis=0)
        xT_ = np.ascontiguousarray(xp.T)                      # [D, S]
        xq_ = np.ascontiguousarray(xT_[:, 0:QH])              # [D, QH]
        # xtt[st, p, dt, s'] = xT[dt*128+p, st*128+s']
        xtt_ = np.ascontiguousarray(
            xT_.reshape(NDT, 128, NST, 128).transpose(2, 1, 0, 3))
        xnb_ = _to_bf16_bits(xp)                              # [S, D]
        in_maps.append({"m12": m12_, "xq": xq_, "xtt": xtt_,
                        "xnb": xnb_, "wvb": wvb_, "lamc": lam_})
    return in_maps


def kernel(x, w_q12, w_k12, w_v, lambda_q1, lambda_k1, lambda_q2, lambda_k2,
           **run_kwargs):
    nc = get_nc()
    in_maps = make_in_maps(x, w_q12, w_k12, w_v, lambda_q1, lambda_k1,
                           lambda_q2, lambda_k2)
    res = run_bass_kernel_spmd(nc, in_maps, list(range(NCORES)), **run_kwargs)
    _CACHE["last_result"] = res
    out = np.empty((B, S, D), dtype=np.float32)
    for c in range(NCORES):
        b, h = divmod(c, 2)
        out[b, h * QH:(h + 1) * QH, :] = res.results[c]["out"]
    return out



# revision 2
# speedup vs baseline: 1.0571x; 1.0571x over previous
"""DiffAttn TRN2 kernel: 8-core SPMD (batch x query-half sharding).

Algebraic restructure vs the direct formulation:

1. Fold the q/k projections into M_a = Wq_a @ Wk_a^T (host, [D, 2D]):
     scores_a = (xq @ M_a) @ x^T          (A-matmul + scores matmul)
   removing the k-projection and all K staging through DRAM.
2. Reassociate the output matmul:
     out = diff @ (x @ Wv) = (diff @ x) @ Wv
   removing the per-core-duplicated v-projection (stage2 t @ Wv is
   only QH*D*D vs the v-projection's S*D*D).
3. The second softmax's lambda weighting uses the per-query ratio
   c[q] = lam * den0[q] / den1[q]:  diff * den0 = e0 - c * e1, and the
   leading 1/den0 is skipped entirely: RMSNorm is scale-invariant per
   row (den0 > 0 so no sign flip). lam is folded on host.
4. Denominators come from an all-ones [128,128] stationary matmul, so
   den lands in PSUM already broadcast across partitions; c is then
   two DVE ops (recip + fused (recip*lam)*den0 scalar_tensor_tensor).

Per core (batch b = core//2, query half h = core%2), x rows permuted
so the core's own q-half comes first (s-order permutation is harmless:
scores/stage1 iterate s-tiles consistently; q rows map to out rows).

  phase 1: A12T[d,q] = sum_din M12[din,d] xqT[din,q]       (f32r)
  phase 3: sT_a[s,q] = sum_d xT[d,s] A_aT[d,q] (xT streamed from HBM
           as host-pre-tiled 4KB-line st-tiles); e_a = exp(scale*s)
           den_a broadcast via ones-matmul; e0 -= c*e1 (DVE bf16)
           stage1: tT[d',q] = sum_s xn[s,d'] e0[s,q]   (bf16, 2
                   concurrent PSUM chains so PE tracks the DVE stream)
           stage2: out[q,d] = sum_d' tT[d',q] wv[d',d] (bf16)
           RMSNorm * (1 - lambda_init)

SBUF: A12T 64K/part resident; x-natural bf16 32K + Wv bf16 16K
resident; e-bf16 32K per q-block; xT st-tiles streamed (24K ring).
~38MB HBM traffic per core, all overlapped.
"""

import sys

for _p in ("/opt/trn_rl_repo", "/root/.axon_site/_ro/trn_rl_repo"):
    if _p not in sys.path:
        sys.path.append(_p)

import numpy as np

import concourse.bass as bass
import concourse.mybir as mybir
from concourse import bacc
from concourse.bass_utils import run_bass_kernel_spmd
from concourse.tile import TileContext

F32 = mybir.dt.float32
F32R = mybir.dt.float32r
BF16 = mybir.dt.bfloat16
U16 = mybir.dt.uint16
AF = mybir.ActivationFunctionType
ALU = mybir.AluOpType

D = 1024          # embed dim
S = 2048          # sequence length
B = 4             # batch
NCORES = 8
QH = 1024         # query rows per core (half a sequence)
QB = 512          # query block (matmul moving dim)
NQB = QH // QB    # 2
NQT = QB // 128   # 4 q-tiles per block
NDT = D // 128    # 8 contraction tiles
NST = S // 128    # 16 key tiles
NMC = 8           # m12 column chunks streamed
MCW = 2 * D // NMC  # 256 columns per chunk
LAMBDA_INIT = 0.8
EPS = 1e-5
SCALE = float(D) ** -0.25

_CACHE = {}


def _build_nc():
    nc = bacc.Bacc("TRN2", target_bir_lowering=False, debug=False,
                   num_devices=NCORES)

    m12 = nc.declare_dram_parameter("m12", [D, 2 * D], U16, isOutput=False)
    xq = nc.declare_dram_parameter("xq", [D, QH], U16, isOutput=False)
    xtt = nc.declare_dram_parameter("xtt", [NST, 128, NDT, 128], U16,
                                    isOutput=False)
    xnb = nc.declare_dram_parameter("xnb", [S, D], U16, isOutput=False)
    wvb = nc.declare_dram_parameter("wvb", [D, D], U16, isOutput=False)
    lamc = nc.declare_dram_parameter("lamc", [128, 1], F32, isOutput=False)
    out = nc.declare_dram_parameter("out", [QH, D], F32, isOutput=True)

    m12_v = m12.ap().rearrange("(dt p) e -> p dt e", p=128).bitcast(BF16)
    xq_v = xq.ap().rearrange("(dt p) q -> p dt q", p=128).bitcast(BF16)
    xtt_v = xtt.ap().bitcast(BF16)                       # [st, p, dt, s']
    xn_v = xnb.ap().rearrange("(st p) e -> p st e", p=128).bitcast(BF16)
    wv_v = wvb.ap().rearrange("(dt p) e -> p dt e", p=128).bitcast(BF16)
    out_v = out.ap().rearrange("(t p) e -> t p e", p=128)   # [8,128,D]

    with TileContext(nc) as tc:
        singles_cm = tc.tile_pool(name="singles", bufs=1)
        singles = singles_cm.__enter__()

        onesq_f = singles.tile([128, 128], F32)
        nc.vector.memset(onesq_f, 1.0)
        onesq_bf = singles.tile([128, 128], BF16)
        nc.vector.tensor_copy(onesq_bf, onesq_f)
        lam_sb = singles.tile([128, 1], F32)
        nc.sync.dma_start(out=lam_sb, in_=lamc.ap())
        eps_sb = singles.tile([128, 1], F32)
        nc.vector.memset(eps_sb, EPS)
        wsink = singles.tile([128, 1], F32)

        # ---- resident tensors --------------------------------------------
        pa12_cm = tc.tile_pool(name="pa12", bufs=1)
        pa12 = pa12_cm.__enter__()
        pxn_cm = tc.tile_pool(name="pxn", bufs=1)
        pxn = pxn_cm.__enter__()
        pwv_cm = tc.tile_pool(name="pwv", bufs=1)
        pwv = pwv_cm.__enter__()

        a12_sb = pa12.tile([128, 2 * NDT, QH], BF16)
        xn_sb = pxn.tile([128, NST, D], BF16)
        wv_sb = pwv.tile([128, NDT, D], BF16)

        # phase-1-only tensors on the right stack (freed after)
        pxq_cm = tc.tile_pool(name="pxq", bufs=1, side="right")
        pxq = pxq_cm.__enter__()
        pm_cm = tc.tile_pool(name="pm", bufs=4, side="right")
        pm = pm_cm.__enter__()
        psa_cm = tc.tile_pool(name="psa", bufs=2, space="PSUM")
        psa = psa_cm.__enter__()

        xq_sb = pxq.tile([128, NDT, QH], BF16)

        # DMA issue order = HBM arrival order. m12 chunk 0 + xq feed the
        # first A-matmul chains; xn/wv/xtt are phase-3 inputs queued later.
        mts = {}
        mts[0] = pm.tile([128, NDT, MCW], BF16, tag="m12", name="mt", bufs=4)
        nc.sync.dma_start(out=mts[0], in_=m12_v[:, :, 0:MCW])
        for qc in range(2):
            for dt in range(NDT):
                nc.sync.dma_start(out=xq_sb[:, dt, qc * 512:(qc + 1) * 512],
                                  in_=xq_v[:, dt, qc * 512:(qc + 1) * 512])
        for mc in range(1, 4):
            mts[mc] = pm.tile([128, NDT, MCW], BF16, tag="m12", name="mt",
                              bufs=4)
            nc.sync.dma_start(out=mts[mc],
                              in_=m12_v[:, :, mc * MCW:(mc + 1) * MCW])

        # warm-up chain: keep the PE busy (and its clock ramped) while the
        # xq/m12 head of the DMA queue lands; sized to the ~13us load time
        pwarm = psa.tile([128, 128], F32, name="pwarm", bufs=1)
        for _ in range(34):
            nc.tensor.matmul(pwarm, lhsT=onesq_bf, rhs=onesq_bf,
                             start=True, stop=True)
        nc.scalar.copy(onesq_f[:, 0:1], pwarm[:, 0:1])

        # ---- phase 1: A12T[d, q] -----------------------------------------
        for mc in range(NMC):
            if mc in mts:
                mt = mts[mc]
            else:
                mt = pm.tile([128, NDT, MCW], BF16, tag="m12", name="mt",
                             bufs=4)
                nc.sync.dma_start(out=mt,
                                  in_=m12_v[:, :, mc * MCW:(mc + 1) * MCW])
            for ti in range(MCW // 128):
                t = mc * (MCW // 128) + ti
                pa = psa.tile([128, QH], F32, name="pa")
                # qc-outer: each 512-half drains while the other computes,
                # so the activation engine never backlogs at phase end
                for qc in range(QH // 512):
                    for dt in range(NDT):
                        nc.tensor.matmul(
                            pa[:, qc * 512:(qc + 1) * 512],
                            lhsT=mt[:, dt, ti * 128:(ti + 1) * 128],
                            rhs=xq_sb[:, dt, qc * 512:(qc + 1) * 512],
                            start=(dt == 0), stop=(dt == NDT - 1))
                    nc.scalar.copy(a12_sb[:, t, qc * 512:(qc + 1) * 512],
                                   pa[:, qc * 512:(qc + 1) * 512])

        # fill the pool-transition bubble (pssc waits on psa's last drain)
        # with dependency-free matmuls so the PE clock stays ramped
        pwarm2 = psa.tile([128, 128], F32, name="pwarm2", bufs=1)
        for _ in range(14):
            nc.tensor.matmul(pwarm2, lhsT=onesq_bf, rhs=onesq_bf,
                             start=True, stop=True)
        nc.scalar.copy(onesq_f[:, 1:2], pwarm2[:, 0:1])

        psa_cm.__exit__(None, None, None)
        pm_cm.__exit__(None, None, None)
        pxq_cm.__exit__(None, None, None)

        # ---- phase 3: attention ------------------------------------------
        # pxtt reuses phase-1 SBUF deliberately: the pool-transition
        # barrier gates its DMAs behind phase-1 completion, so the 24MB of
        # phase-3 streams cannot steal HBM bandwidth from the phase-1
        # critical path (DMA queues execute concurrently, not in program
        # order).
        with tc.tile_pool(name="pxtt", bufs=6) as pxtt, \
             tc.tile_pool(name="eblk", bufs=1) as eblk, \
             tc.tile_pool(name="work", bufs=2) as work, \
             tc.tile_pool(name="pssc", bufs=2, space="PSUM") as pssc, \
             tc.tile_pool(name="psden", bufs=1, space="PSUM") as psden, \
             tc.tile_pool(name="psout", bufs=1, space="PSUM") as psout:
            for bi in range(NQB):
                qs = bi * QB
                eT = {}
                pden = {}
                for a in (0, 1):
                    eT[a] = eblk.tile([128, NST, QB], BF16,
                                      tag=f"e{a}", name=f"eT{a}")
                    pden[a] = psden.tile([128, QB], F32, tag=f"den{a}",
                                         name=f"pden{a}")
                for st in range(NST):
                    xt = pxtt.tile([128, NDT, 128], BF16, tag="xtt",
                                   name="xt", bufs=6)
                    nc.sync.dma_start(out=xt, in_=xtt_v[st])
                    if bi == 0 and st == 5:
                        for dt in range(NDT):
                            nc.sync.dma_start(out=wv_sb[:, dt, :],
                                              in_=wv_v[:, dt, :])
                        for st2 in range(NST):
                            nc.sync.dma_start(out=xn_sb[:, st2, :],
                                              in_=xn_v[:, st2, :])
                    for a in (0, 1):
                        psc = pssc.tile([128, QB], F32, tag="sc", name="psc")
                        for dt in range(NDT):
                            nc.tensor.matmul(
                                psc,
                                lhsT=xt[:, dt, :],
                                rhs=a12_sb[:, a * NDT + dt, qs:qs + QB],
                                start=(dt == 0), stop=(dt == NDT - 1))
                        nc.scalar.activation(eT[a][:, st, :], psc, AF.Exp,
                                             scale=SCALE)
                    # den ones-matmuls pipelined one st behind the score
                    # chains (their exps have completed by then). The
                    # all-ones [128,128] lhsT broadcasts den over all
                    # partitions for free.
                    if st > 0:
                        for a in (0, 1):
                            nc.tensor.matmul(
                                pden[a], lhsT=onesq_bf,
                                rhs=eT[a][:, st - 1, :],
                                start=(st - 1 == 0), stop=False)
                for a in (0, 1):
                    nc.tensor.matmul(pden[a], lhsT=onesq_bf,
                                     rhs=eT[a][:, NST - 1, :],
                                     start=False, stop=True)
                # fill the den -> combine dependency bubble with
                # dependency-free matmuls (keeps the PE clock ramped)
                pwf = pssc.tile([128, QB], F32, tag="sc", name="warmf")
                for _ in range(20):
                    nc.tensor.matmul(pwf[:, 0:128], lhsT=onesq_bf,
                                     rhs=onesq_bf, start=True, stop=True)
                nc.scalar.copy(wsink, pwf[:, 0:1])
                # c[q] = lam * den0[q] / den1[q]; e0 <- e0 - c*e1.
                # 1/den0 is never applied: RMSNorm cancels per-row scales.
                rden = work.tile([128, QB], F32, tag="rden", name="rden",
                                 bufs=1)
                nc.vector.reciprocal_approx_fast(rden, pden[1])
                c_bf = work.tile([128, QB], BF16, tag="cbf", name="cbf",
                                 bufs=1)
                nc.vector.scalar_tensor_tensor(
                    c_bf, rden, lam_sb, pden[0],
                    op0=ALU.mult, op1=ALU.mult)
                for st in range(NST):
                    nc.vector.tensor_mul(eT[1][:, st, :], eT[1][:, st, :],
                                         c_bf)
                    nc.vector.tensor_sub(eT[0][:, st, :], eT[0][:, st, :],
                                         eT[1][:, st, :])
                # stage1: tT[d', q] = sum_s xn[s, d'] e0[s, q], two d'
                # chains at a time (852ns/st consumption tracks the DVE
                # combine stream), 4 passes over the e tiles
                tT = work.tile([128, NDT, QB], BF16, tag="tt", name="tT",
                               bufs=1)
                # alternate PSUM tag pairs (A/B) per pass so a pass's
                # chains never wait on the previous pass's drains
                for dp in range(NDT // 2):
                    ab = "AB"[dp % 2]
                    pt = {}
                    for k in (0, 1):
                        pt[k] = psout.tile([128, QB], F32, tag=f"p{ab}{k}",
                                           name=f"pt{k}")
                    for st in range(NST):
                        for k in (0, 1):
                            dpt = dp * 2 + k
                            nc.tensor.matmul(
                                pt[k],
                                lhsT=xn_sb[:, st, dpt * 128:(dpt + 1) * 128],
                                rhs=eT[0][:, st, :],
                                start=(st == 0), stop=(st == NST - 1))
                    for k in (0, 1):
                        nc.scalar.copy(tT[:, dp * 2 + k, :], pt[k])
                # stage2: out[q, d] = sum_d' tT[d', q] wv[d', d] + RMSNorm.
                # Square runs per-half straight off PSUM (no copy on the
                # critical path); the final scale also reads PSUM directly.
                for j in range(NQT):
                    ab = "AB"[j % 2]
                    pos = {}
                    ssqs = {}
                    for dh in range(2):
                        po = psout.tile([128, 512], F32, tag=f"p{ab}{dh}",
                                        name=f"po{dh}")
                        for dpt in range(NDT):
                            nc.tensor.matmul(
                                po,
                                lhsT=tT[:, dpt, j * 128:(j + 1) * 128],
                                rhs=wv_sb[:, dpt, dh * 512:(dh + 1) * 512],
                                start=(dpt == 0), stop=(dpt == NDT - 1))
                        pos[dh] = po
                        sqv = work.tile([128, 512], BF16, tag="sq",
                                        name="sqv", bufs=1)
                        ssqs[dh] = work.tile([128, 1], F32, tag=f"ssq{dh}",
                                             name=f"ssq{dh}")
                        nc.scalar.activation(sqv, po, AF.Square,
                                             accum_out=ssqs[dh])
                    nc.scalar.add(ssqs[0], ssqs[0], ssqs[1])
                    rms = work.tile([128, 1], F32, tag="rms", name="rms")
                    nc.scalar.activation(rms, ssqs[0], AF.Sqrt,
                                         scale=1.0 / D, bias=eps_sb)
                    rr = work.tile([128, 1], F32, tag="rr", name="rr")
                    nc.vector.reciprocal(rr, rms)
                    outs = work.tile([128, D], F32, tag="outs", name="outs",
                                     bufs=3)
                    for dh in range(2):
                        nc.vector.tensor_scalar(
                            outs[:, dh * 512:(dh + 1) * 512], pos[dh], rr,
                            1.0 - LAMBDA_INIT, op0=ALU.mult, op1=ALU.mult)
                        nc.sync.dma_start(
                            out=out_v[bi * NQT + j][:, dh * 512:(dh + 1) * 512],
                            in_=outs[:, dh * 512:(dh + 1) * 512])

        pwv_cm.__exit__(None, None, None)
        pxn_cm.__exit__(None, None, None)
        pa12_cm.__exit__(None, None, None)
        singles_cm.__exit__(None, None, None)

    nc.finalize()
    return nc


def get_nc():
    if "nc" not in _CACHE:
        _CACHE["nc"] = _build_nc()
    return _CACHE["nc"]


def _to_bf16_bits(a):
    u = np.ascontiguousarray(a, dtype=np.float32).view(np.uint32)
    return (((u >> 16) + ((u >> 15) & 1)).astype(np.uint32) & 0xFFFF).astype(
        np.uint16)


def make_in_maps(x, w_q12, w_k12, w_v, lambda_q1, lambda_k1, lambda_q2,
                 lambda_k2):
    wq = np.asarray(w_q12, dtype=np.float64)
    wk = np.asarray(w_k12, dtype=np.float64)
    m1 = wq[:, :D] @ wk[:, :D].T
    m2 = wq[:, D:] @ wk[:, D:].T
    m12_ = _to_bf16_bits(
        np.concatenate([m1, m2], axis=1).astype(np.float32))
    wvb_ = _to_bf16_bits(np.asarray(w_v, dtype=np.float32))
    lam1 = np.exp(np.float64(lambda_q1) @ np.float64(lambda_k1))
    lam2 = np.exp(np.float64(lambda_q2) @ np.float64(lambda_k2))
    lam_ = np.full((128, 1), lam1 - lam2 + LAMBDA_INIT, dtype=np.float32)
    in_maps = []
    for c in range(NCORES):
        b, h = divmod(c, 2)
        xb = np.asarray(x[b], dtype=np.float32)
        # own q-half rows first so the kernel's q columns are 0:QH
        xp = np.concatenate([xb[h * QH:(h + 1) * QH, :],
                             xb[(1 - h) * QH:(2 - h) * QH, :]], axis=0)
        xT_ = np.ascontiguousarray(xp.T)                      # [D, S]
        xq_ = _to_bf16_bits(xT_[:, 0:QH])                     # [D, QH]
        # xtt[st, p, dt, s'] = xT[dt*128+p, st*128+s']
        xtt_ = _to_bf16_bits(
            xT_.reshape(NDT, 128, NST, 128).transpose(2, 1, 0, 3))
        xnb_ = _to_bf16_bits(xp)                              # [S, D]
        in_maps.append({"m12": m12_, "xq": xq_, "xtt": xtt_,
                        "xnb": xnb_, "wvb": wvb_, "lamc": lam_})
    return in_maps


def kernel(x, w_q12, w_k12, w_v, lambda_q1, lambda_k1, lambda_q2, lambda_k2,
           **run_kwargs):
    nc = get_nc()
    in_maps = make_in_maps(x, w_q12, w_k12, w_v, lambda_q1, lambda_k1,
                           lambda_q2, lambda_k2)
    res = run_bass_kernel_spmd(nc, in_maps, list(range(NCORES)), **run_kwargs)
    _CACHE["last_result"] = res
    out = np.empty((B, S, D), dtype=np.float32)
    for c in range(NCORES):
        b, h = divmod(c, 2)
        out[b, h * QH:(h + 1) * QH, :] = res.results[c]["out"]
    return out

